# revision 4
# baseline (speedup 1.0000x reference)
"""GRU-D cell kernel for Trainium2 (8 NeuronCores, data-parallel over batch).

Strategy
--------
Data-parallel: batch (16384) split 8 ways -> 2048 rows/core, weights
replicated. Everything on-chip is feature-major (transposed) so matmul
contractions run with the contraction dim on SBUF partitions and zero
on-chip transposes. The batch runs in 4 chunks of 512 columns; chunk
c+1's DMA overlaps chunk c's gate matmuls.

Precision / perf config (numpy-sim validated; sim tracks HW relmax to
~5%): the PE sustains ~2.4GHz at ANY fp8-DoubleRow duty (prior
"power-cap downclock" readings were cold-chip clock-ramp artifacts), so
every matmul whose fp8 noise passes the 2e-2 relmax gate runs fp8-DR:
gamma_x, the mask thirds of r/z/i_n (acts {0,16} exact in e4m3), the
r-gate xt+h thirds, and h_n (acts cast on-device to e4m3 x16 by the
scalar engine; noise damped through sigmoid*h_n*tanh). z xt/h thirds,
i_n xt third, and gamma_h stay bf16 — fp8 there measured 1.9-2.7e-2,
over the gate. PE cycles: 9.25 units (v1) -> 7.5 = 983K cycles/core.

A uniform gate-PSUM product scale x1024 lets fp8 (act x16 * w x64) and
bf16 (act x1 * w x1024; 2^10 exact in bf16) thirds share a PSUM; gate
activations descale via their scale parameter. Host prep is
elementwise-only (A = m*x+(1-m)*mu, D = (1-m)*(xl-mu), transposes,
dtype packs); x_t = A + dx*D and h = dh*hs are computed on-device.

DMA layout: act streams are restrided host-side to [chunk, partition,
ktile, col] so each chunk load is one contiguous slab per partition
(4-8KB lines, ~2x the per-queue rate of a feature-major gather); the
six per-j gate packs are merged into one fp8 + one bf16 slab. Chunk-0's
sync ring orders the gamma packs contiguously with j0's packs directly
behind (wgx, wg, j0, j1..) so neither gamma's dh half nor the first
gate group is pack-starved; routing any of this via the idle gpsimd
queue measured strictly worse (+14us) — its dispatch path is slow.
Output is fp16 (upcast host-side; +5e-6 relmax).

Measured: 452.4us warm @ relmax 1.6014e-2 (v1 baseline 549.7us @
1.264e-2; the first run after device-open/reset or a fresh NEFF lands
+40-100us from clock ramp). ~407us PE at 2.4GHz + ~16us DVFS ramp over
the first 50us + ~10us pre + ~5us gaps + ~10us out tail + ~9us drain.
"""

import os
from contextlib import ExitStack

import numpy as np
import ml_dtypes

import concourse.bass as bass
import concourse.mybir as mybir
import concourse.tile as tile
from concourse import bacc
from concourse.bass import ds
from concourse.bass_utils import run_bass_kernel_spmd

BF16 = mybir.dt.bfloat16
F32 = mybir.dt.float32
F8E4 = mybir.dt.float8e4
F16O = mybir.dt.float16
NPBF = ml_dtypes.bfloat16
NPF8 = ml_dtypes.float8_e4m3

P = 128
E = 1024
B = 16384
NCORES = 8
BC = B // NCORES
NB = 512
KE = E // P        # 8
MG = 2 * E // P    # 16
JT = E // P        # 8

AF = mybir.ActivationFunctionType
PM = mybir.MatmulPerfMode
WSC = 4096.0       # gamma_x fp8 weight scale
DSC = 16.0         # delta / act fp8 scale
GSC = 64.0         # gate fp8 weight scale
BSC = 1024.0       # bf16 gate weight scale (acts true-scale; 2^10 exact)
DESC = 1.0 / (DSC * GSC)   # 1/1024 gate psum descale

HN_FP8 = True      # False: hn third bf16 at x1 (own psum scale)
Z_H_FP8 = False    # False: z h-third bf16 (w x1024 @ true-scale h)

LAST_EXEC_NS = None
LAST_RESULTS = None


def build_gru_d(bc=BC, nb=NB):
    nch = bc // nb
    nc = bacc.Bacc("TRN2", target_bir_lowering=False)

    # act streams restrided host-side to [chunk, partition, ktile, col]:
    # every chunk load is one fully-contiguous slab per partition (4-8KB
    # lines vs the 0.5-1KB of a feature-major gather) -> ~2-3x per-queue
    # DMA rate, which is what bounds the chunk-0 head window.
    nch_ = bc // NB
    dT = nc.declare_dram_parameter("dT", [nch_, P, KE, NB], BF16, isOutput=False)
    dT8 = nc.declare_dram_parameter("dT8", [nch_, P, KE, NB], F8E4, isOutput=False)
    mT8 = nc.declare_dram_parameter("mT8", [nch_, P, KE, NB], F8E4, isOutput=False)
    # planes: A, D, hs (true scale)
    xlmh = nc.declare_dram_parameter("xlmh", [3, nch_, P, KE, NB], BF16,
                                     isOutput=False)
    # gamma packs pre-transposed host-side to partition-major
    wgx = nc.declare_dram_parameter("wgx", [P, KE, KE, P], F8E4, isOutput=False)
    wg = nc.declare_dram_parameter("wg", [P, KE, KE * P], BF16, isOutput=False)
    # gate packs merged per dtype: ONE fp8 + ONE bf16 DMA per j.
    # fp8 k-tile layout: [r: mask|xt|h (24) | z-mask (8) | in-mask (8) | hn (8)]
    # bf16 (x1024 weights): [z-xt (8) | z-h (8) | in-xt (8)]
    w8a = nc.declare_dram_parameter("w8a", [JT, P, 6 * KE, P], F8E4,
                                    isOutput=False)
    w16a = nc.declare_dram_parameter("w16a", [JT, P, 3 * KE, P], BF16,
                                     isOutput=False)
    biases = nc.declare_dram_parameter("biases", [P, 6 * JT], F32, isOutput=False)
    outT = nc.declare_dram_parameter("outT", [E, bc], F16O, isOutput=True)

    with ExitStack() as ctx:
        tc = ctx.enter_context(tile.TileContext(nc))
        p_bias = ctx.enter_context(tc.tile_pool(name="bias", bufs=1))
        p_psum = ctx.enter_context(tc.tile_pool(name="psum", bufs=8, space="PSUM"))
        p_act = ctx.enter_context(tc.tile_pool(name="acts", bufs=2))
        p_xth = ctx.enter_context(tc.tile_pool(name="xth", bufs=1))
        p_pk = ctx.enter_context(tc.tile_pool(name="pack", bufs=1))
        p_wg = ctx.enter_context(tc.tile_pool(name="wgp", bufs=3))
        p_w3 = ctx.enter_context(tc.tile_pool(name="w3p", bufs=2))
        p_g = ctx.enter_context(tc.tile_pool(name="gp", bufs=16))
        p_tmp = ctx.enter_context(tc.tile_pool(name="tmp", bufs=6))
        p_gact = ctx.enter_context(tc.tile_pool(name="gact", bufs=5))
        p_ep = ctx.enter_context(tc.tile_pool(name="ep", bufs=8))
        p_out = ctx.enter_context(tc.tile_pool(name="outp", bufs=4))

        bias_sb = p_bias.tile([P, 6 * JT], F32)
        nc.sync.dma_start(out=bias_sb, in_=biases[:, :])
        OB_G, OB_RZ, OB_NN, OB_HN = 0, MG, MG + 2 * JT, MG + 3 * JT

        for c in range(nch):
            cs = ds(c * nb, nb)
            # ---- chunk loads (same two-ring split as v1: acts on the
            # scalar queue, weight packs on the sync queue) ----
            dT8_c = p_act.tile([P, KE, nb], F8E4, tag="dT8c")
            nc.scalar.dma_start(out=dT8_c[:, ds(0, 2), :], in_=dT8[c, :, ds(0, 2), :])
            nc.scalar.dma_start(out=dT8_c[:, ds(2, 6), :], in_=dT8[c, :, ds(2, 6), :])
            dT_c = p_act.tile([P, KE, nb], BF16, tag="dTc")
            nc.scalar.dma_start(out=dT_c[:, 0, :], in_=dT[c, :, 0, :])
            nc.scalar.dma_start(out=dT_c[:, ds(1, 7), :], in_=dT[c, :, ds(1, 7), :])
            wgx_c = p_wg.tile([P, KE, KE, P], F8E4, tag="wgxall", bufs=1)
            nc.sync.dma_start(out=wgx_c[:, ds(0, 2), :, :], in_=wgx[:, ds(0, 2)])
            nc.sync.dma_start(out=wgx_c[:, ds(2, 6), :, :], in_=wgx[:, ds(2, 6)])
            wg_c = p_wg.tile([P, KE, KE * P], BF16, tag="wgall", bufs=1)
            nc.sync.dma_start(out=wg_c[:, ds(0, 4), :], in_=wg[:, ds(0, 4)])
            nc.sync.dma_start(out=wg_c[:, ds(4, 4), :], in_=wg[:, ds(4, 4)])
            if c == 0:
                # j0's gate packs directly behind the (contiguous) gamma
                # packs on the sync ring: wg-b lands ~6us earlier than the
                # interleaved order, j0's packs still make their ~+27us
                # (ramped-clock) first use
                w8_j0 = p_w3.tile([P, 6 * KE, P], F8E4, tag="w8a", bufs=3)
                nc.sync.dma_start(out=w8_j0, in_=w8a[0])
                w16_j0 = p_w3.tile([P, 3 * KE, P], BF16, tag="w16a", bufs=3)
                nc.sync.dma_start(out=w16_j0, in_=w16a[0])
            mT8_c = p_act.tile([P, KE, nb], F8E4, tag="mTc")
            nc.scalar.dma_start(out=mT8_c, in_=mT8[c])
            xl_c = p_pk.tile([P, 3, KE, nb], BF16, tag="xlmh")
            nc.scalar.dma_start(out=xl_c, in_=xlmh[:, c].rearrange("t p k b -> p t k b"))
            xt_c = p_xth.tile([P, KE, nb], BF16, tag="xtc")
            xt8_c = p_xth.tile([P, KE, nb], F8E4, tag="xt8")
            h_c = p_xth.tile([P, KE, nb], BF16, tag="hc")
            h8_c = p_xth.tile([P, KE, nb], F8E4, tag="h8")

            # ---- gamma + prologue (x_t/h/casts hidden under gamma MMs) ----
            prologue = []
            for mi in range(MG):
                ps = p_psum.tile([P, nb], F32, tag="ps")
                if mi < KE:  # dx: fp8 DoubleRow
                    for t in range(KE // 2):
                        nc.tensor.matmul(
                            ps, wgx_c[:, mi, ds(2 * t, 2), :],
                            dT8_c[:, ds(2 * t, 2), :],
                            start=(t == 0), stop=(t == KE // 2 - 1),
                            perf_mode=PM.DoubleRow,
                        )
                    scl = -1.0 / (WSC * DSC)
                else:  # dh: bf16
                    for k in range(KE):
                        nc.tensor.matmul(
                            ps, wg_c[:, mi - KE, ds(k * P, P)], dT_c[:, k, :],
                            start=(k == 0), stop=(k == KE - 1),
                        )
                    scl = -1.0
                e_t = p_tmp.tile([P, nb], BF16, tag="et", bufs=4)
                nc.scalar.activation(e_t, ps, AF.Exp, scale=scl,
                                     bias=bias_sb[:, ds(OB_G + mi, 1)])
                g_t = p_g.tile([P, nb], BF16, tag="g")
                nc.vector.tensor_scalar_min(g_t, e_t, 1.0)

                def emit_prologue(mi=mi, g_t=g_t):
                    if mi < KE:
                        j = mi  # xt[j] = A[j] + dx[j]*D[j]
                        t1 = p_tmp.tile([P, nb], BF16, tag="xtmp", name="t1", bufs=2)
                        nc.vector.tensor_mul(t1, g_t, xl_c[:, 1, j, :])
                        nc.vector.tensor_add(xt_c[:, j, :], t1, xl_c[:, 0, j, :])
                        nc.scalar.activation(xt8_c[:, j, :], xt_c[:, j, :],
                                             AF.Copy, scale=DSC)
                    else:
                        j = mi - KE  # h[j] = dh[j]*hs[j] (true scale)
                        nc.vector.tensor_mul(h_c[:, j, :], g_t, xl_c[:, 2, j, :])
                        nc.scalar.activation(h8_c[:, j, :], h_c[:, j, :],
                                             AF.Copy, scale=16.0)

                if c == 0:
                    prologue.append(emit_prologue)
                else:
                    emit_prologue()
            for fn in prologue:
                fn()

            # ---- gates: group order r, hn, in, z ----
            for j in range(JT):
                if c == 0 and j == 0:
                    w8_t, w16_t = w8_j0, w16_j0
                else:
                    w8_t = p_w3.tile([P, 6 * KE, P], F8E4, tag="w8a", bufs=3)
                    nc.sync.dma_start(out=w8_t, in_=w8a[j])
                    w16_t = p_w3.tile([P, 3 * KE, P], BF16, tag="w16a", bufs=3)
                    nc.sync.dma_start(out=w16_t, in_=w16a[j])
                w_r = w8_t            # ktiles 0..23: r mask|xt|h
                w_z8 = w8_t           # ktiles 24..31: z mask (offset 3*KE)
                w_im = w8_t           # ktiles 32..39: in mask (offset 4*KE)
                w_h = w8_t            # ktiles 40..47: hn      (offset 5*KE)

                def dr_run(ps, w, wofs, act, first, last):
                    for t in range(KE // 2):
                        nc.tensor.matmul(
                            ps, w[:, ds(wofs + 2 * t, 2), :],
                            act[:, ds(2 * t, 2), :],
                            start=(first and t == 0),
                            stop=(last and t == KE // 2 - 1),
                            perf_mode=PM.DoubleRow)

                # r: 12 DR matmuls (mask, xt, h)
                ps = p_psum.tile([P, nb], F32, tag="ps")
                dr_run(ps, w_r, 0, mT8_c, True, False)
                dr_run(ps, w_r, KE, xt8_c, False, False)
                dr_run(ps, w_r, 2 * KE, h8_c, False, True)
                r_t = p_gact.tile([P, nb], BF16, tag="rt", bufs=4)
                nc.scalar.activation(r_t, ps, AF.Sigmoid, scale=DESC,
                                     bias=bias_sb[:, ds(OB_RZ + j, 1)])

                # hn
                ps = p_psum.tile([P, nb], F32, tag="ps")
                if HN_FP8:
                    dr_run(ps, w_h, 5 * KE, h8_c, True, True)
                    hn_scl = DESC
                else:
                    for kk in range(KE):
                        nc.tensor.matmul(ps, w_h[:, ds(kk * P, P)], h_c[:, kk, :],
                                         start=(kk == 0), stop=(kk == KE - 1))
                    hn_scl = 1.0
                hnb_t = p_gact.tile([P, nb], BF16, tag="hnbt", bufs=4)
                nc.scalar.activation(hnb_t, ps, AF.Identity, scale=hn_scl,
                                     bias=bias_sb[:, ds(OB_HN + j, 1)])

                # i_n: fp8 mask DR + bf16 xt
                ps_in = p_psum.tile([P, nb], F32, tag="ps", name="ps_in")
                dr_run(ps_in, w_im, 4 * KE, mT8_c, True, False)
                for kk in range(KE):
                    nc.tensor.matmul(ps_in, w16_t[:, 2 * KE + kk, :],
                                     xt_c[:, kk, :],
                                     start=False, stop=(kk == KE - 1))
                in_t = p_ep.tile([P, nb], F32, tag="eptmp", name="in_t")
                nc.scalar.activation(in_t, ps_in, AF.Identity, scale=DESC,
                                     bias=bias_sb[:, ds(OB_NN + j, 1)])
                # n = tanh(in_t + r*hnb);  out = n + z*(h - n)
                t_m = p_ep.tile([P, nb], F32, tag="eptmp")
                nc.vector.tensor_mul(t_m, r_t, hnb_t)
                u_t = p_ep.tile([P, nb], F32, tag="eptmp")
                nc.vector.tensor_add(u_t, t_m, in_t)
                n_t = p_ep.tile([P, nb], F32, tag="eptmp")
                nc.scalar.activation(n_t, u_t, AF.Tanh)
                hm_t = p_ep.tile([P, nb], F32, tag="eptmp")
                nc.vector.tensor_sub(hm_t, h_c[:, j, :], n_t)

                # z: fp8 mask DR + bf16 xt (+ h: fp8 DR or bf16 x1024)
                ps = p_psum.tile([P, nb], F32, tag="ps")
                dr_run(ps, w_z8, 3 * KE, mT8_c, True, False)
                for kk in range(KE):
                    nc.tensor.matmul(ps, w16_t[:, kk, :], xt_c[:, kk, :],
                                     start=False, stop=False)
                for kk in range(KE):
                    nc.tensor.matmul(ps, w16_t[:, KE + kk, :], h_c[:, kk, :],
                                     start=False, stop=(kk == KE - 1))
                if c == nch - 1 and j == JT - 1:
                    for s in range(2):
                        sl = ds(s * (nb // 2), nb // 2)
                        z_s = p_gact.tile([P, nb // 2], BF16, tag="zts", bufs=2)
                        nc.scalar.activation(z_s, ps[:, sl], AF.Sigmoid,
                                             scale=DESC,
                                             bias=bias_sb[:, ds(OB_RZ + JT + j, 1)])
                        zm_s = p_ep.tile([P, nb // 2], F32, tag="epsl", bufs=2)
                        nc.vector.tensor_mul(zm_s, z_s, hm_t[:, sl])
                        o_s = p_out.tile([P, nb // 2], F16O, tag="ots", bufs=2)
                        nc.vector.tensor_add(o_s, n_t[:, sl], zm_s)
                        nc.scalar.dma_start(
                            out=outT[ds(j * P, P), ds(c * nb + s * (nb // 2), nb // 2)],
                            in_=o_s)
                else:
                    z_t = p_gact.tile([P, nb], BF16, tag="zt", bufs=4)
                    nc.scalar.activation(z_t, ps, AF.Sigmoid, scale=DESC,
                                         bias=bias_sb[:, ds(OB_RZ + JT + j, 1)])
                    zm_t = p_ep.tile([P, nb], F32, tag="eptmp")
                    nc.vector.tensor_mul(zm_t, z_t, hm_t)
                    o_t = p_out.tile([P, nb], F16O, tag="ot", bufs=3)
                    nc.vector.tensor_add(o_t, n_t, zm_t)
                    nc.scalar.dma_start(out=outT[ds(j * P, P), cs], in_=o_t)
    nc.compile()
    return nc


def prep_shared(inputs):
    gxw, gxb = inputs["gx_w"], inputs["gx_b"]
    ghw, ghb = inputs["gh_w"], inputs["gh_b"]
    wih, whh = inputs["w_ih"], inputs["w_hh"]
    bih, bhh = inputs["b_ih"], inputs["b_hh"]

    def pack(w, dt=NPBF, scale=1.0):
        # [K, M] -> [m_tiles, P, k_tiles, P]  (value = w[k*P+p, m*P+c])
        K, M = w.shape
        return np.ascontiguousarray(
            w.reshape(K // P, P, M // P, P).transpose(2, 1, 0, 3) * scale
        ).astype(dt)

    WgT = np.concatenate([gxw, ghw], axis=0).T          # [E, 2E] lhsT

    def colpk(v):
        return v.reshape(-1, P).T

    bias_pk = np.concatenate([
        colpk(-np.concatenate([gxb, ghb])),
        colpk((bih + bhh)[: 2 * E]),
        colpk(bih[2 * E:]),
        colpk(bhh[2 * E:]),
    ], axis=1).astype(np.float32)
    # gamma packs pre-transposed to partition-major [P, m, k(, c)]
    wgx_p = pack(WgT[:, :E], dt=NPF8, scale=WSC).transpose(1, 0, 2, 3)
    wg_p = pack(WgT[:, E:]).transpose(1, 0, 2, 3).reshape(P, E // P, E)
    # merged per-j packs (one fp8 + one bf16 DMA per gate-feature tile):
    # fp8 ktiles [r: mask|xt|h (24) | z-mask (8) | in-mask (8) | hn (8)]
    w8a = np.concatenate([
        pack(np.concatenate([wih[E: 2 * E, :E], wih[:E, :E],
                             whh[:, :E]], axis=0), dt=NPF8, scale=GSC),
        pack(np.ascontiguousarray(wih[E: 2 * E, E: 2 * E]), dt=NPF8, scale=GSC),
        pack(np.ascontiguousarray(wih[E: 2 * E, 2 * E:]), dt=NPF8, scale=GSC),
        pack(np.ascontiguousarray(whh[:, 2 * E:]), dt=NPF8, scale=GSC),
    ], axis=2)
    # bf16 x1024 ktiles [z-xt (8) | z-h (8) | in-xt (8)]
    w16a = np.concatenate([
        pack(np.concatenate([wih[:E, E: 2 * E], whh[:, E: 2 * E]], axis=0),
             scale=BSC),
        pack(np.ascontiguousarray(wih[:E, 2 * E:]), scale=BSC),
    ], axis=2)
    shared = {
        "wgx": np.ascontiguousarray(wgx_p),
        "wg": np.ascontiguousarray(wg_p),
        "w8a": np.ascontiguousarray(w8a),
        "w16a": np.ascontiguousarray(w16a),
        "biases": np.ascontiguousarray(bias_pk),
    }
    return shared


def prep_core(inputs, rows, shared):
    msk = inputs["x_mask"][rows]
    x = inputs["x"][rows]
    mu = inputs["x_mean"][rows]
    xl = inputs["x_last_observed"][rows]
    A = msk * x + (1.0 - msk) * mu
    D = (1.0 - msk) * (xl - mu)
    def cm(a):
        # [bc, E] row-major -> [nch, P, KE, NB] chunk/partition-major so
        # each chunk DMA reads one contiguous slab per partition
        return np.ascontiguousarray(
            a.T.reshape(E // P, P, BC // NB, NB).transpose(2, 1, 0, 3))

    m = {
        "dT": cm(inputs["delta"][rows].astype(NPBF)),
        "dT8": cm((inputs["delta"][rows] * DSC).astype(NPF8)),
        "mT8": cm((msk * DSC).astype(NPF8)),   # {0,16}: exact in e4m3
        "xlmh": np.stack([
            cm(A.astype(NPBF)),
            cm(D.astype(NPBF)),
            cm(inputs["hs"][rows].astype(NPBF)),
        ]),
    }
    m.update(shared)
    return m


def kernel(**inputs):
    global LAST_EXEC_NS, LAST_RESULTS
    inputs = {k: np.asarray(v) for k, v in inputs.items()}
    nc = build_gru_d(BC, NB)
    shared = prep_shared(inputs)
    in_maps = [
        prep_core(inputs, slice(i * BC, (i + 1) * BC), shared) for i in range(NCORES)
    ]
    trace = bool(os.environ.get("GRUD_TRACE"))
    res = run_bass_kernel_spmd(nc, in_maps, list(range(NCORES)), trace=trace)
    LAST_RESULTS = res
    LAST_EXEC_NS = res.exec_time_ns
    out = np.empty((B, E), np.float32)
    for i in range(NCORES):
        out[i * BC : (i + 1) * BC] = res.results[i]["outT"].T.astype(np.float32)
    return out
